# revision 1
# baseline (speedup 1.0000x reference)
# Multi-head attention with RoPE, tensor-parallel over heads on 8 NeuronCores.
#
# Problem: B=2, N=2048, D=1024, H=16 heads, head_dim=64.
#   q/k/v = x @ W{q,k,v}.T + b;  RoPE(q), RoPE(k);  softmax(q k^T / 8) v;
#   out = attn @ Wo.T + bo.
#
# Sharding: 2 heads per core (column-parallel QKV, row-parallel out-proj).
# Each core emits a partial [T, D] output; host sums the 8 partials.
#
# Per-core layout strategy (all matmul inputs bf16, fp32 accumulation):
#   xT   [D, T]    : x transposed on host, so the contraction dim d is on
#                    partitions for every projection matmul.
#   q,k  [E=128, T]: "transposed" activations (2 heads * 64 on partitions).
#   RoPE           : rotate-half done as a 128x128 constant permutation matmul
#                    (rot = P @ q), then q'' = q*cos + rot*sin on DVE.
#   v    [T, E]    : natural layout; per (head, tk-chunk) tiles [128, 65]
#                    with a ones column so attn@v also yields softmax row sums.
#   scores^T       : [tk, tq] via lhsT=k (so exp output feeds attn@v directly
#                    as the moving operand, free dim 512).
#   attn@v         : oT[65, tq] = [v | 1]^T @ expT, accumulated over tk in
#                    PSUM; row 64 = softmax denominator.
#   normalize      : recip of row 64, broadcast to 128 partitions via a K=1
#                    matmul, multiplied into oT during the PSUM->SBUF copy.
#   out-proj       : y[t, :] += oT_h^T @ WoT_h per head (K=64), plus bo on
#                    core 0 only (other cores receive zeros for bo).

import numpy as np
import ml_dtypes

import concourse.bass as bass
import concourse.mybir as mybir
import concourse.tile as tile
from concourse import bacc

B, N, D, H = 2, 2048, 1024, 16
HD = 64
T = B * N                 # 4096 tokens
NCORES = 8
HPC = H // NCORES         # 2 heads per core
E = HPC * HD              # 128 per-core projection columns
KD = D // 128             # 8 contraction tiles for d
ROPE_BASE = 10000.0

BF = mybir.dt.bfloat16
F32 = mybir.dt.float32
F32R = mybir.dt.float32r

TQC = 1024                # tq chunk (exp granularity / psum width)
NTQC = N // TQC           # 2 per batch


def build_nc():
    nc = bacc.Bacc(trn_type="TRN2", target_bir_lowering=False, debug=False)

    xT = nc.dram_tensor("xT", [D, T], BF, kind="ExternalInput").ap()
    wqT = nc.dram_tensor("wqT", [D, E], BF, kind="ExternalInput").ap()
    wkT = nc.dram_tensor("wkT", [D, E], BF, kind="ExternalInput").ap()
    wvT = nc.dram_tensor("wvT", [D, E], BF, kind="ExternalInput").ap()
    woT = nc.dram_tensor("woT", [E, D], BF, kind="ExternalInput").ap()
    brow = nc.dram_tensor("brow", [1, 3 * E], BF, kind="ExternalInput").ap()
    borow = nc.dram_tensor("borow", [1, D], BF, kind="ExternalInput").ap()
    cosb = nc.dram_tensor("cosb", [E, N], BF, kind="ExternalInput").ap()
    sinb = nc.dram_tensor("sinb", [E, N], BF, kind="ExternalInput").ap()
    rotT = nc.dram_tensor("rotT", [E, E], BF, kind="ExternalInput").ap()
    ones1 = nc.dram_tensor("ones1", [1, TQC], F32R, kind="ExternalInput").ap()
    y = nc.dram_tensor("y", [T, D], F32, kind="ExternalOutput").ap()

    with tile.TileContext(nc) as tc:
        _build(tc, nc, xT, wqT, wkT, wvT, woT, brow, borow, cosb, sinb,
               rotT, ones1, y)
    nc.compile()  # bacc legalization: splits multi-wait instructions etc.
    return nc


def _build(tc, nc, xT, wqT, wkT, wvT, woT, brow, borow, cosb, sinb,
           rotT, ones1, y):
    with (
        tc.tile_pool(name="consts", bufs=1) as consts,
        tc.tile_pool(name="xbig", bufs=1) as xbig,
        tc.tile_pool(name="acts", bufs=1) as acts,
        tc.tile_pool(name="small", bufs=3) as small,
    ):
        # ---- constants / weights ----
        wq_sb = consts.tile([128, KD, E], BF, tag="wq")
        wk_sb = consts.tile([128, KD, E], BF, tag="wk")
        wv_sb = consts.tile([128, KD, E], BF, tag="wv")
        nc.sync.dma_start(out=wq_sb, in_=wqT.rearrange("(k p) e -> p k e", p=128))
        nc.sync.dma_start(out=wk_sb, in_=wkT.rearrange("(k p) e -> p k e", p=128))
        nc.sync.dma_start(out=wv_sb, in_=wvT.rearrange("(k p) e -> p k e", p=128))
        # [64, head, D] so each head's slice sits at base partition 0
        wo_sb = consts.tile([HD, HPC, D], BF, tag="wo")
        nc.sync.dma_start(out=wo_sb, in_=woT.rearrange("(h e) d -> e h d", h=HPC))
        # biases live as [1, n] rows and are folded into each matmul
        # accumulation group as a K=1 rank-1 update (ones^T @ bias_row),
        # so no post-matmul elementwise bias add is needed anywhere.
        brow_sb = consts.tile([1, 3 * E], BF, tag="brow")
        nc.sync.dma_start(out=brow_sb, in_=brow)
        borow_sb = consts.tile([1, D], BF, tag="borow")
        nc.sync.dma_start(out=borow_sb, in_=borow)
        onesr_sb = consts.tile([1, 512], BF, tag="onesr")
        nc.vector.memset(onesr_sb, 1.0)
        cos_sb = consts.tile([E, N], BF, tag="cos")
        sin_sb = consts.tile([E, N], BF, tag="sin")
        nc.sync.dma_start(out=cos_sb, in_=cosb)
        nc.sync.dma_start(out=sin_sb, in_=sinb)
        rot_sb = consts.tile([E, E], BF, tag="rot")
        nc.sync.dma_start(out=rot_sb, in_=rotT)
        ones1_sb = consts.tile([1, TQC], F32R, tag="ones1")
        nc.sync.dma_start(out=ones1_sb, in_=ones1)

        # ---- x^T resident (reused later for expT) ----
        # 8 t-sliced DMAs so the first q/k matmuls start after ~1/8 of x.
        x_sb = xbig.tile([128, KD, T], BF, tag="big")
        xTr = xT.rearrange("(k p) t -> p k t", p=128)
        for ci in range(T // 512):
            nc.sync.dma_start(out=x_sb[:, :, ci * 512:(ci + 1) * 512],
                              in_=xTr[:, :, ci * 512:(ci + 1) * 512])

        # ---- persistent activations ----
        q_sb = acts.tile([E, T], BF, tag="q_sb")
        k_sb = acts.tile([E, T], BF, tag="k_sb")
        q2 = acts.tile([E, T], BF, tag="q2")
        k2 = acts.tile([E, T], BF, tag="k2")
        # v tiles: [tk 128, tk-chunk 32, head 2, 64+ones]
        v_sb = acts.tile([128, T // 128, HPC, HD + 1], BF, tag="v_sb")
        # normalized attention output, transposed: [e 64, (b,h) 4, tq 2048]
        on_sb = acts.tile([HD, B * HPC, N], BF, tag="on_sb")

        nc.vector.memset(v_sb[:, :, :, HD:HD + 1], 1.0)

        # ================= phase 1: projections + rope =================
        with (
            tc.tile_pool(name="ps_qk", bufs=2, space="PSUM") as ps_qk,
            tc.tile_pool(name="ps_v", bufs=2, space="PSUM") as ps_v,
            tc.tile_pool(name="ps_r", bufs=2, space="PSUM") as ps_r,
        ):
            # Emission interleaved per 512-token slice so compute tracks the
            # incoming x DMA stream and the PE never sits idle long.
            for ci in range(T // 512):
                sl = slice(ci * 512, (ci + 1) * 512)
                npos = (ci * 512) % N
                tsl = slice(npos, npos + 512)
                # q / k projection + rope for this slice
                for dst, w, bcol, dst2 in ((q_sb, wq_sb, 0, q2),
                                           (k_sb, wk_sb, 1, k2)):
                    ps = ps_qk.tile([128, 512], F32, tag="ps_qk")
                    for k in range(KD):
                        nc.tensor.matmul(
                            ps, w[:, k, :], x_sb[:, k, sl],
                            start=(k == 0), stop=False)
                    nc.tensor.matmul(
                        ps, brow_sb[:, bcol * E:(bcol + 1) * E], onesr_sb,
                        start=False, stop=True)
                    nc.vector.tensor_copy(dst[:, sl], ps)
                    # rope: dst2 = dst*cos + (P@dst)*sin
                    psr = ps_r.tile([128, 512], F32, tag="ps_r")
                    nc.tensor.matmul(psr, rot_sb, dst[:, sl],
                                     start=True, stop=True)
                    t1 = small.tile([128, 512], BF, tag="rope_t1")
                    nc.vector.tensor_mul(t1, dst[:, sl], cos_sb[:, tsl])
                    t2 = small.tile([128, 512], BF, tag="rope_t2")
                    nc.vector.tensor_mul(t2, psr, sin_sb[:, tsl])
                    nc.vector.tensor_add(dst2[:, sl], t1, t2)
                # v for the four 128-token sub-chunks of this slice
                for s in range(4):
                    cv = ci * 4 + s
                    psv = ps_v.tile([128, E], F32, tag="ps_v")
                    for k in range(KD):
                        nc.tensor.matmul(
                            psv, x_sb[:, k, cv * 128:(cv + 1) * 128],
                            wv_sb[:, k, :], start=(k == 0), stop=False)
                    nc.tensor.matmul(
                        psv, onesr_sb[:, 0:128], brow_sb[:, 2 * E:3 * E],
                        start=False, stop=True)
                    for h in range(HPC):
                        nc.vector.tensor_copy(
                            v_sb[:, cv, h, 0:HD], psv[:, h * HD:(h + 1) * HD])

        # ========= phase 2+3: attention + output projection =========
        # Per (b, tqc): for each key chunk tkc, scores (both heads,
        # row-packed) -> exp -> attn@v MMs, interleaved so the PE always has
        # attn work for chunk tkc while ACT computes exp for chunk tkc+1.
        # Each batch's output projection is emitted right after its second
        # tq chunk, drawing PSUM from the scores pool (scores idle then).
        with (
            tc.tile_pool(name="ps_sc", bufs=2, space="PSUM") as ps_sc,
            tc.tile_pool(name="ps_o", bufs=2, space="PSUM") as ps_o,
        ):
            # Deferred-work queue: each block's normalization chain (whose
            # 6.5us single-partition reciprocal would otherwise stall the
            # PE at the block boundary) and the per-batch output-projection
            # chunks are emitted one-per-tkc inside LATER blocks' loops, so
            # the scores/exp/attn drum never pauses.
            pending = []

            def norm_item(b, tqc, h, ou, rs):
                # ou: unnormalized attn output [64, TQC] bf16 (SBUF)
                # rs: softmax denominators [1, TQC] f32r (SBUF)
                def emit():
                    rc = small.tile([1, TQC], F32R, tag="recip", bufs=2,
                                    name=f"rc_{b}_{tqc}_{h}")
                    with nc.allow_low_precision(reason="f32r == f32 bits"):
                        nc.vector.reciprocal(rc, rs)
                    rb = ps_sc.tile([128, TQC], F32, tag="ps_sc",
                                    name=f"rb_{b}_{tqc}_{h}")
                    for nn in range(TQC // 512):
                        nc.tensor.matmul(
                            rb[:, nn * 512:(nn + 1) * 512], ones1_sb[:, 0:128],
                            rc[:, nn * 512:(nn + 1) * 512],
                            start=True, stop=True)
                    rbs = small.tile([128, TQC], BF, tag="recipb", bufs=2,
                                     name=f"rbs_{b}_{tqc}_{h}")
                    nc.vector.tensor_copy(rbs, rb)
                    nc.vector.tensor_mul(
                        on_sb[:, b * HPC + h, tqc * TQC:(tqc + 1) * TQC],
                        ou, rbs[0:HD, :])
                    if h == HPC - 1:
                        # both heads of (b, tqc) normalized -> the matching
                        # output-projection chunks are now eligible
                        for ci in range(tqc * 8, tqc * 8 + 8):
                            pending.append(y_item(b, ci))
                return emit

            def y_item(b, ci):
                def emit():
                    psy = ps_sc.tile([128, D], F32, tag="ps_sc",
                                     name=f"psy_{b}_{ci}")
                    for eo in range(D // 512):
                        po = psy[:, eo * 512:(eo + 1) * 512]
                        for h in range(HPC):
                            nc.tensor.matmul(
                                po,
                                on_sb[:, b * HPC + h, ci * 128:(ci + 1) * 128],
                                wo_sb[:, h, eo * 512:(eo + 1) * 512],
                                start=(h == 0), stop=False)
                        nc.tensor.matmul(
                            po, onesr_sb[:, 0:128],
                            borow_sb[:, eo * 512:(eo + 1) * 512],
                            start=False, stop=True)
                    ysb = small.tile([128, D], F32, tag="ysb", bufs=2,
                                     name=f"ysb_{b}_{ci}")
                    if ci % 2 == 0:
                        nc.vector.tensor_copy(ysb, psy)
                    else:
                        nc.scalar.copy(ysb, psy)
                    nc.sync.dma_start(
                        out=y[b * N + ci * 128: b * N + (ci + 1) * 128, :],
                        in_=ysb)
                return emit

            for b in range(B):
                for tqc in range(NTQC):
                    tq0 = b * N + tqc * TQC  # global tq base
                    exp_t = xbig.tile([128, HPC, N // 128, TQC], BF, tag="big")
                    ots = [ps_o.tile([HD + 1, TQC], F32, tag="ps_o",
                                     name=f"ot_{b}_{tqc}_{h}")
                           for h in range(HPC)]
                    def attn_mms(j):
                        # attn@v for key chunk j (consumes exp_t[:, :, j, :])
                        for h in range(HPC):
                            vt = v_sb[:, b * (N // 128) + j, h, :]
                            for nn in range(TQC // 512):
                                nc.tensor.matmul(
                                    ots[h][:, nn * 512:(nn + 1) * 512], vt,
                                    exp_t[:, h, j, nn * 512:(nn + 1) * 512],
                                    start=(j == 0), stop=(j == N // 128 - 1))

                    # Software-pipelined: the PE's attn@v for chunk tkc-1 is
                    # emitted after exp(tkc) so the PE never waits on the
                    # same-iteration exp; ACT (exp) is the steady-state drum.
                    # One deferred item (prev block's normalization / y-proj
                    # chunk) is woven in per tkc iteration.
                    for tkc in range(N // 128):
                        scs = []
                        for h in range(HPC):
                            sc = ps_sc.tile([128, TQC], F32, tag="ps_sc")
                            lhsT = k2[h * HD:(h + 1) * HD,
                                      b * N + tkc * 128: b * N + (tkc + 1) * 128]
                            for nn in range(TQC // 512):
                                nc.tensor.matmul(
                                    sc[:, nn * 512:(nn + 1) * 512], lhsT,
                                    q2[h * HD:(h + 1) * HD,
                                       tq0 + nn * 512: tq0 + (nn + 1) * 512],
                                    start=True, stop=True)
                            scs.append(sc)
                        for h in range(HPC):
                            nc.scalar.activation(
                                out=exp_t[:, h, tkc, :], in_=scs[h],
                                func=mybir.ActivationFunctionType.Exp,
                                scale=float(HD) ** -0.5)
                        if tkc > 0:
                            attn_mms(tkc - 1)
                        if tkc >= 1 and pending:
                            pending.pop(0)()
                    attn_mms(N // 128 - 1)

                    # Evacuate the ot PSUM tiles quickly (two cheap copies
                    # each) so the next block's attn matmuls get the slots;
                    # the expensive reciprocal is deferred via `pending`.
                    for h in range(HPC):
                        ou = small.tile([HD, TQC], BF, tag="ou", bufs=4,
                                        name=f"ou_{b}_{tqc}_{h}")
                        nc.vector.tensor_copy(ou, ots[h][0:HD, :])
                        rs = small.tile([1, TQC], F32R, tag="rs", bufs=4,
                                        name=f"rs_{b}_{tqc}_{h}")
                        with nc.allow_low_precision(reason="f32r == f32 bits"):
                            nc.vector.tensor_copy(rs, ots[h][HD:HD + 1, :])
                        pending.append(norm_item(b, tqc, h, ou, rs))

            # drain remaining deferred work (last block's norms + final ys)
            while pending:
                pending.pop(0)()


def _host_inputs(x, Wq, Wk, Wv, Wo, bq, bk, bv, bo):
    """Build the 8 per-core input maps (host-side sharding + layout prep)."""
    bf16 = ml_dtypes.bfloat16
    xTh = np.ascontiguousarray(x.reshape(T, D).T).astype(bf16)

    # rope tables: row e uses freq (e % 64) % 32; positions along columns
    i = (np.arange(E) % HD) % (HD // 2)
    inv_freq = ROPE_BASE ** (-2.0 * i / HD)  # [E]
    ang = np.arange(N)[None, :] * inv_freq[:, None]          # [E, N]
    cosb = np.cos(ang).astype(bf16)
    sinb = np.sin(ang).astype(bf16)

    # rotate-half permutation: rot = P @ q (per 64-block)
    P = np.zeros((E, E), dtype=np.float32)
    for h in range(HPC):
        for j in range(HD // 2):
            P[h * HD + j, h * HD + j + HD // 2] = -1.0
            P[h * HD + j + HD // 2, h * HD + j] = 1.0
    rotT = np.ascontiguousarray(P.T).astype(bf16)

    ones1 = np.ones((1, TQC), dtype=np.float32)

    in_maps = []
    for c in range(NCORES):
        sl = slice(c * E, (c + 1) * E)
        borow = (bo[None, :] if c == 0 else
                 np.zeros((1, D))).astype(bf16)
        in_maps.append({
            "xT": xTh,
            "wqT": np.ascontiguousarray(Wq[sl, :].T).astype(bf16),
            "wkT": np.ascontiguousarray(Wk[sl, :].T).astype(bf16),
            "wvT": np.ascontiguousarray(Wv[sl, :].T).astype(bf16),
            "woT": np.ascontiguousarray(Wo[:, sl].T).astype(bf16),
            "brow": np.concatenate([bq[sl], bk[sl], bv[sl]])[None, :]
                .astype(bf16),
            "borow": borow,
            "cosb": cosb,
            "sinb": sinb,
            "rotT": rotT,
            "ones1": ones1,
        })
    return in_maps


_NC = None


def kernel(x, Wq, Wk, Wv, Wo, bq, bk, bv, bo):
    from concourse.bass_utils import run_bass_kernel_spmd

    global _NC
    if _NC is None:
        _NC = build_nc()
    in_maps = _host_inputs(np.asarray(x, dtype=np.float32),
                           np.asarray(Wq, dtype=np.float32),
                           np.asarray(Wk, dtype=np.float32),
                           np.asarray(Wv, dtype=np.float32),
                           np.asarray(Wo, dtype=np.float32),
                           np.asarray(bq, dtype=np.float32),
                           np.asarray(bk, dtype=np.float32),
                           np.asarray(bv, dtype=np.float32),
                           np.asarray(bo, dtype=np.float32))
    res = run_bass_kernel_spmd(_NC, in_maps, core_ids=list(range(NCORES)))
    out = np.zeros((T, D), dtype=np.float32)
    for r in res.results:
        out += r["y"]
    return out.reshape(B, N, D)

